# revision 14
# baseline (speedup 1.0000x reference)
"""Trainium2 Bass kernel for nn_CtoX (gnn_message_passing).

Computes, per batch b:
  stage1 (CtoE): block-pair stats (mean/min/max/std with pairwise masks) of
     delta1[b] over 16x16 atom blocks -> z[b, 16, 16, 256] -> E = z @ W1.T + b1
  stage2 (EtoX): masked stats of E over its second block axis -> zE[b,16,256]
     -> out = zE @ W2.T + b2   (out: [4, 16, 256])

Sharding: 8 cores = (4 batches) x (2 halves of the first nm axis).
Each core handles delta1[b, ih*128:(ih+1)*128, :, :] (8 MiB) and produces
out[b, ih*8:(ih+1)*8, :] with zero cross-core communication.

v3 layout notes:
  - The host pre-transposes each core's delta1 slice to [128 i, J, c, a]
    (c outer, a inner) so that on-device the bias-add and the grouped
    min/max reduce over `a` are all inner-contiguous (2 elem/cycle DVE
    path) and the per-J DMA lands contiguously (1 descriptor/partition).
  - Per-J working tile dsq[P, 128, MA]: columns 0:64 = d (DMA),
    64:128 = d^2 (Scalar Square).  The sums matmul uses the dsq column
    for one `a` as the STATIONARY tensor and the 16 indicator columns
    [bind*cm1*cm2[j] | bind] as the MOVING tensor, so the accumulated
    result S_psT[(c|c^2), J, (Sm I | S1 I)] comes out with features on
    partitions -- no stage-2 sum transposes needed at all.
  - Bias-adds run on GpSimd for most J (Vector keeps all reduces since
    GpSimd can't reduce over free axes); all mask-derived constants are
    host-precomputed into one cpack tensor.
"""

import numpy as np
from contextlib import ExitStack

BIG = 100000.0
EPS = 1e-8

D_C = 64      # channel dim of delta1
D_X = 256     # output feature dim
MA = 16       # atoms per block
P = 128       # partitions per core (half of nm)
NI = 8        # I-blocks per core
NJ = 16       # J-blocks
NM = 256

GMIN = 8      # J < GMIN: min-path bias-add on GpSimd (else Vector).
              # Max-path bias-adds all run on GpSimd.

# cpack column offsets (one packed [128, CPACK_COLS] constants tensor)
# -- section A (loop-critical, first DMA) --
OFF_LHST = 0                       # [256*16] lhsT fields per j
OFF_BIASF = OFF_LHST + NM * 16     # [256] BIG*(1-cm1*cm2)
OFF_BIASFN = OFF_BIASF + NM        # [256] -BIG*(1-cm1*cm2)
A_COLS = OFF_BIASFN + NM
# -- section B (stage-2, second DMA) --
OFF_IDENT = A_COLS                 # [128]
OFF_RECIPD = OFF_IDENT + P         # [16*8] 1/(cnt1*cnt2+eps), (J,I) order
OFF_NFAC = OFF_RECIPD + NJ * NI    # [16*8] 1-EPS/div, (J,I) order
OFF_W1T = OFF_NFAC + NJ * NI       # [256]
OFF_W2T = OFF_W1T + 256            # [512]
OFF_B1 = OFF_W2T + 512             # [1]
OFF_B2 = OFF_B1 + 1                # [2]
OFF_EM = OFF_B2 + 2                # [16]
OFF_BIASE = OFF_EM + NJ            # [16]
OFF_BIASEN = OFF_BIASE + NJ        # [16]
OFF_RECIPE = OFF_BIASEN + NJ       # [1]
CPACK_COLS = OFF_RECIPE + 1

_CACHE = {}


def _build_program():
    import concourse.bass as bass
    import concourse.bacc as bacc
    import concourse.tile as tile
    import concourse.mybir as mybir

    f32 = mybir.dt.float32
    Alu = mybir.AluOpType
    Act = mybir.ActivationFunctionType
    AX = mybir.AxisListType

    nc = bacc.Bacc()

    # d, host-pretransposed: [i, J, c, a] flattened to [128, NJ, D_C*MA]
    d_in = nc.dram_tensor("d", [P, NJ, D_C * MA], f32, kind="ExternalInput")
    cpack_in = nc.dram_tensor("cpack", [P, CPACK_COLS], f32, kind="ExternalInput")
    out_t = nc.dram_tensor("out_t", [D_X, NI], f32, kind="ExternalOutput")

    with tile.TileContext(nc) as tc, ExitStack() as ctx:
        consts = ctx.enter_context(tc.tile_pool(name="consts", bufs=1))
        small = ctx.enter_context(tc.tile_pool(name="small", bufs=1))

        # ---------- constant loads: two DMAs (loop-critical part first) ----
        cpak = consts.tile([P, CPACK_COLS], f32)
        nc.sync.dma_start(out=cpak[:, 0:A_COLS], in_=cpack_in[:, 0:A_COLS])
        nc.sync.dma_start(
            out=cpak[:, A_COLS:CPACK_COLS], in_=cpack_in[:, A_COLS:CPACK_COLS]
        )
        lhsTs = cpak[:, OFF_LHST : OFF_LHST + NM * 16].rearrange(
            "p (j k) -> p j k", k=16
        )
        biasF = cpak[:, OFF_BIASF : OFF_BIASF + NM]
        biasFn = cpak[:, OFF_BIASFN : OFF_BIASFN + NM]
        ident = cpak[:, OFF_IDENT : OFF_IDENT + P]
        recipD = cpak[:, OFF_RECIPD : OFF_RECIPD + NJ * NI].rearrange(
            "p (J I) -> p J I", I=NI
        )
        nfac = cpak[:, OFF_NFAC : OFF_NFAC + NJ * NI].rearrange(
            "p (J I) -> p J I", I=NI
        )
        w1t_a = cpak[:, OFF_W1T : OFF_W1T + 128]
        w1t_b = cpak[:, OFF_W1T + 128 : OFF_W1T + 256]
        w2t_a = cpak[:, OFF_W2T : OFF_W2T + 256]
        w2t_b = cpak[:, OFF_W2T + 256 : OFF_W2T + 512]
        b1c = cpak[:, OFF_B1 : OFF_B1 + 1]
        b2c_a = cpak[:, OFF_B2 : OFF_B2 + 1]
        b2c_b = cpak[:, OFF_B2 + 1 : OFF_B2 + 2]
        emrep = cpak[:, OFF_EM : OFF_EM + NJ]
        biasE = cpak[:, OFF_BIASE : OFF_BIASE + NJ]
        biasEn = cpak[:, OFF_BIASEN : OFF_BIASEN + NJ]
        recipE = cpak[:, OFF_RECIPE : OFF_RECIPE + 1]

        # ---------- big J-loop ----------
        # umm[:, J, 0:64] = per-(i, J, c) biased max; [:, J, 64:128] = biased
        # min -- packed so ONE transpose per J lands ma-feats at partitions
        # 0:64 and mi-feats at 64:128.
        umm = consts.tile([P, NJ, P], f32)
        # z matrices in [feature, row=(I,J)] layout:
        #   rhs_z0: [0:64] m-feats, [64:128] mi-feats
        #   rhs_z1: [0:64] ma-feats, [64:128] std-feats
        rhs_z0 = small.tile([P, P], f32)
        rhs_z1 = small.tile([P, P], f32)
        # evacuated sums, features on partitions: [(c|c2), J, (Sm I | S1 I)]
        SS = small.tile([P, NJ, 16], f32)

        with tc.tile_pool(name="psum_sums", bufs=1, space="PSUM") as psum_sums, \
             tc.tile_pool(name="psum_tr", bufs=2, space="PSUM") as psum_tr, \
             tc.tile_pool(name="loop", bufs=3) as loop_pool, \
             tc.tile_pool(name="btmp", bufs=3) as btmp_pool, \
             tc.tile_pool(name="gtmp", bufs=3) as gtmp_pool:
            S_psT = psum_sums.tile([P, NJ, 16], f32)

            for J in range(NJ):
                # packed [d | d^2] tile, layout [i, c2, a] (a contiguous)
                dsq = loop_pool.tile([P, P, MA], f32, tag="dsq")
                nc.sync.dma_start(
                    out=dsq[:, 0:64, :].rearrange("p c a -> p (c a)"),
                    in_=d_in[:, J, :],
                )
                nc.scalar.activation(
                    out=dsq[:, 64:128, :], in_=dsq[:, 0:64, :], func=Act.Square
                )

                # min path: bias-add on GpSimd for J < GMIN, else Vector
                menge = nc.gpsimd if J < GMIN else nc.vector
                mpool = gtmp_pool if J < GMIN else btmp_pool
                bt = mpool.tile([P, D_C, MA], f32, tag="bt")
                menge.tensor_tensor(
                    out=bt,
                    in0=dsq[:, 0:64, :],
                    in1=biasF[:, J * MA : (J + 1) * MA]
                    .unsqueeze(1)
                    .broadcast_to([P, D_C, MA]),
                    op=Alu.add,
                )
                nc.vector.tensor_reduce(
                    out=umm[:, J, 64:128],
                    in_=bt[:],
                    axis=AX.X,
                    op=Alu.min,
                )
                # max path: bias-add on GpSimd for all J
                bt2 = gtmp_pool.tile([P, D_C, MA], f32, tag="bt2")
                nc.gpsimd.tensor_tensor(
                    out=bt2,
                    in0=dsq[:, 0:64, :],
                    in1=biasFn[:, J * MA : (J + 1) * MA]
                    .unsqueeze(1)
                    .broadcast_to([P, D_C, MA]),
                    op=Alu.add,
                )
                nc.vector.tensor_reduce(
                    out=umm[:, J, 0:64],
                    in_=bt2[:],
                    axis=AX.X,
                    op=Alu.max,
                )

                # sums: ONE matmul per j.  STATIONARY = dsq column for this
                # a (128 rows = [d c | d^2 c]), MOVING = 16 indicator cols
                # [bind*cm1*cm2[j] | bind].  out[f, k] = sum_i dsq[i,f]*w[i,k]
                # accumulated over jj: partitions 0:64 = per-c sums,
                # 64:128 = per-c d^2 sums; free 0:8 = masked (Sm/S2),
                # 8:16 = plain (S1).
                for jj in range(MA):
                    j = J * MA + jj
                    nc.tensor.matmul(
                        S_psT[:, J, :],
                        lhsT=dsq[:, :, jj],
                        rhs=lhsTs[:, j, :],
                        start=(jj == 0),
                        stop=(jj == MA - 1),
                    )

            # evacuate sums (features already on partitions)
            nc.scalar.copy(SS[:], S_psT[:])

            # min/max stage 2: one transpose per J into a PSUM ring of 4,
            # then ONE batched grouped reduce per 4-J round per feature-half.
            for Jr in range(0, NJ, 4):
                TP = psum_tr.tile([P, 4, P], f32, tag="tp")
                for k in range(4):
                    nc.tensor.transpose(
                        out=TP[:, k, :], in_=umm[:, Jr + k, :], identity=ident
                    )
                nc.vector.tensor_reduce(
                    out=rhs_z0[64:128, :]
                    .rearrange("p (I J) -> p J I", J=NJ)[:, Jr : Jr + 4, :],
                    in_=TP[64:128, :, :].rearrange("p r (I a) -> p r I a", a=MA),
                    axis=AX.X,
                    op=Alu.min,
                )
                nc.vector.tensor_reduce(
                    out=rhs_z1[0:64, :]
                    .rearrange("p (I J) -> p J I", J=NJ)[:, Jr : Jr + 4, :],
                    in_=TP[0:64, :, :].rearrange("p r (I a) -> p r I a", a=MA),
                    axis=AX.X,
                    op=Alu.max,
                )

        # ---------- stage 2: stats algebra in [feature, (J, I)] layout ----
        with tc.tile_pool(name="psum_e", bufs=1, space="PSUM") as psum_e, \
             tc.tile_pool(name="psum_o", bufs=1, space="PSUM") as psum_o:
            # SS layout: partitions 0:64 = c (from d cols), 64:128 = c (from
            # d^2 cols); free k 0:8 = masked sums (Sm lo / S2 hi), 8:16 =
            # plain sums (S1 lo / junk hi).  The std algebra on hi
            # partitions also needs Sm and S1 -- shift them up with one
            # SBUF->SBUF partition-offset DMA.
            SS2 = small.tile([P, NJ, 16], f32)
            nc.sync.dma_start(out=SS2[64:128, :, :], in_=SS[0:64, :, :])

            # m = S1/div  (lo half -> m-features; hi half feeds std)
            mT = small.tile([P, NJ, NI], f32)
            nc.vector.tensor_tensor(
                out=mT[0:64], in0=SS[0:64, :, 8:16], in1=recipD[0:64],
                op=Alu.mult,
            )
            nc.vector.tensor_tensor(
                out=mT[64:128], in0=SS2[64:128, :, 8:16], in1=recipD[64:128],
                op=Alu.mult,
            )
            nc.vector.tensor_copy(
                out=rhs_z0[0:64, :].rearrange("p (I J) -> p J I", J=NJ),
                in_=mT[0:64],
            )
            # std = S2/div - 2*m*(Sm/div) + m^2*nfac     (hi half only)
            A = small.tile([P, NJ, NI], f32)
            nc.vector.tensor_tensor(
                out=A[64:128], in0=SS[64:128, :, 0:8], in1=recipD[64:128],
                op=Alu.mult,
            )
            Bq = small.tile([P, NJ, NI], f32)
            nc.vector.tensor_tensor(
                out=Bq[64:128], in0=SS2[64:128, :, 0:8], in1=recipD[64:128],
                op=Alu.mult,
            )
            nc.vector.tensor_tensor(
                out=Bq[64:128], in0=Bq[64:128], in1=mT[64:128], op=Alu.mult
            )
            nc.vector.tensor_scalar(
                Bq[64:128], Bq[64:128], -2.0, None, Alu.mult
            )  # -2*m*Sm/div
            nc.vector.tensor_tensor(
                out=A[64:128], in0=A[64:128], in1=Bq[64:128], op=Alu.add
            )
            Cq = small.tile([P, NJ, NI], f32)
            nc.vector.tensor_tensor(
                out=Cq[64:128], in0=mT[64:128], in1=mT[64:128], op=Alu.mult
            )
            nc.vector.tensor_tensor(
                out=Cq[64:128], in0=Cq[64:128], in1=nfac[64:128], op=Alu.mult
            )
            nc.vector.tensor_tensor(
                out=rhs_z1[64:128, :].rearrange("p (I J) -> p J I", J=NJ),
                in0=A[64:128],
                in1=Cq[64:128],
                op=Alu.add,
            )

            # ---------- E = z @ W1.T + b1 (dup channels on 128 parts) ----
            E_ps = psum_e.tile([P, P], f32)
            nc.tensor.matmul(
                E_ps[:], lhsT=w1t_a, rhs=rhs_z0[:], start=True, stop=False
            )
            nc.tensor.matmul(
                E_ps[:], lhsT=w1t_b, rhs=rhs_z1[:], start=False, stop=True
            )
            E_T = small.tile([P, P], f32)  # [128(dup chan), 128 rows=(I,J)]
            nc.scalar.activation(
                out=E_T[:], in_=E_ps[:], func=Act.Identity, bias=b1c, scale=1.0
            )

            # ---------- stage 2 of the net: masked stats over J ----------
            E_r = E_T[:].rearrange("p (I J) -> p I J", J=NJ)
            zE0 = small.tile([P, NI], f32)  # [0:64] mE, [64:128] miE
            zE1 = small.tile([P, NI], f32)  # [0:64] maE, [64:128] stdE

            # mE (all partitions; lo half is the m-feature, hi feeds stdE)
            mE = small.tile([P, NI], f32)
            nc.vector.tensor_reduce(out=mE[:], in_=E_r, axis=AX.X, op=Alu.add)
            nc.scalar.mul(mE[:], mE[:], recipE)
            nc.scalar.copy(zE0[0:64, :], mE[0:64, :])

            # miE on hi half
            bE = small.tile([P, NI, NJ], f32)
            nc.gpsimd.tensor_tensor(
                out=bE[64:128],
                in0=E_r[64:128],
                in1=biasE[64:128].unsqueeze(1).broadcast_to([64, NI, NJ]),
                op=Alu.add,
            )
            nc.vector.tensor_reduce(
                out=zE0[64:128, :], in_=bE[64:128], axis=AX.X, op=Alu.min
            )
            # maE on lo half (Vector)
            bE2 = small.tile([P, NI, NJ], f32)
            nc.vector.tensor_tensor(
                out=bE2[0:64],
                in0=E_r[0:64],
                in1=biasEn[0:64].unsqueeze(1).broadcast_to([64, NI, NJ]),
                op=Alu.add,
            )
            nc.vector.tensor_reduce(
                out=zE1[0:64, :], in_=bE2[0:64], axis=AX.X, op=Alu.max
            )
            # stdE on hi half: sum(em*(E-mE)^2)/denom  (GpSimd)
            dev = small.tile([P, NI, NJ], f32)
            nc.gpsimd.tensor_tensor(
                out=dev[64:128],
                in0=E_r[64:128],
                in1=mE[64:128].unsqueeze(2).broadcast_to([64, NI, NJ]),
                op=Alu.subtract,
            )
            nc.gpsimd.tensor_tensor(
                out=dev[64:128], in0=dev[64:128], in1=dev[64:128], op=Alu.mult
            )
            nc.gpsimd.tensor_tensor(
                out=dev[64:128],
                in0=dev[64:128],
                in1=emrep[64:128].unsqueeze(1).broadcast_to([64, NI, NJ]),
                op=Alu.mult,
            )
            nc.vector.tensor_reduce(
                out=zE1[64:128, :], in_=dev[64:128], axis=AX.X, op=Alu.add
            )
            nc.scalar.mul(zE1[64:128, :], zE1[64:128, :], recipE[64:128])

            # ---------- out = zE @ W2.T + b2 ----------
            outa_ps = psum_o.tile([128, NI], f32)
            outb_ps = psum_o.tile([128, NI], f32)
            nc.tensor.matmul(
                outa_ps[:], lhsT=w2t_a[:, 0:128], rhs=zE0[:], start=True, stop=False
            )
            nc.tensor.matmul(
                outa_ps[:], lhsT=w2t_b[:, 0:128], rhs=zE1[:], start=False, stop=True
            )
            nc.tensor.matmul(
                outb_ps[:], lhsT=w2t_a[:, 128:256], rhs=zE0[:], start=True, stop=False
            )
            nc.tensor.matmul(
                outb_ps[:], lhsT=w2t_b[:, 128:256], rhs=zE1[:], start=False,
                stop=True,
            )
            outa = small.tile([128, NI], f32)
            nc.scalar.activation(
                out=outa[:], in_=outa_ps[:], func=Act.Identity, bias=b2c_a,
                scale=1.0,
            )
            outb = small.tile([128, NI], f32)
            nc.scalar.activation(
                out=outb[:], in_=outb_ps[:], func=Act.Identity, bias=b2c_b,
                scale=1.0,
            )
            nc.sync.dma_start(out=out_t[0:128, :], in_=outa[:])
            nc.sync.dma_start(out=out_t[128:256, :], in_=outb[:])

    nc.finalize()  # Bacc: runs compile() (wait splitting, reg alloc, ...)
    return nc


def _get_program():
    if "nc" not in _CACHE:
        _CACHE["nc"] = _build_program()
    return _CACHE["nc"]


def _make_in_maps(delta1, c_mask1, c_mask2, e_mask2, W1, b1, W2, b2):
    delta1 = np.asarray(delta1, dtype=np.float32)
    c_mask1 = np.asarray(c_mask1, dtype=np.float32)
    c_mask2 = np.asarray(c_mask2, dtype=np.float32)
    e_mask2 = np.asarray(e_mask2, dtype=np.float32)
    W1 = np.asarray(W1, dtype=np.float32)
    b1 = np.asarray(b1, dtype=np.float32)
    W2 = np.asarray(W2, dtype=np.float32)
    b2 = np.asarray(b2, dtype=np.float32)

    w1t = np.concatenate([W1.T, W1.T], axis=1)  # [256, 128] (dup out-chan)
    w2t = W2.T  # [256, 256]
    bindm = np.zeros((128, 8), dtype=np.float32)
    for i in range(128):
        bindm[i, i // 16] = 1.0
    identm = np.eye(128, dtype=np.float32)

    in_maps = []
    for k in range(8):
        b, ih = k // 2, k % 2
        cm1 = c_mask1[b, ih * 128 : (ih + 1) * 128, 0, 0]        # [128]
        cm2 = c_mask2[b, 0, :, 0]                                 # [256]
        em = e_mask2[b, 0, :, 0]                                  # [16]

        # pre-transpose d to [i, J, c, a]
        dslab = delta1[b, ih * 128 : (ih + 1) * 128]              # [128,256,64]
        dT = np.ascontiguousarray(
            dslab.reshape(128, NJ, MA, D_C).transpose(0, 1, 3, 2)
        ).reshape(128, NJ, D_C * MA)

        cp = np.zeros((128, CPACK_COLS), dtype=np.float32)
        # lhsT fields: [128, 256 j, 16]: cols 0:8 bind*cm1*cm2[j], 8:16 bind
        lhst = np.zeros((128, NM, 16), dtype=np.float32)
        lhst[:, :, 0:8] = (
            bindm[:, None, :] * cm1[:, None, None] * cm2[None, :, None]
        )
        lhst[:, :, 8:16] = bindm[:, None, :]
        cp[:, OFF_LHST : OFF_LHST + NM * 16] = lhst.reshape(128, NM * 16)
        t2 = cm1[:, None] * cm2[None, :]                          # [128, 256]
        cp[:, OFF_BIASF : OFF_BIASF + NM] = BIG * (1.0 - t2)
        cp[:, OFF_BIASFN : OFF_BIASFN + NM] = -BIG * (1.0 - t2)
        cp[:, OFF_IDENT : OFF_IDENT + P] = identm
        cnt1 = bindm.T @ cm1                                      # [8]
        cnt2 = cm2.reshape(NJ, MA).sum(axis=1)                    # [16]
        div = cnt2[:, None] * cnt1[None, :] + EPS                 # [16 J, 8 I]
        cp[:, OFF_RECIPD : OFF_RECIPD + NJ * NI] = (1.0 / div).reshape(-1)[None, :]
        cp[:, OFF_NFAC : OFF_NFAC + NJ * NI] = (1.0 - EPS / div).reshape(-1)[None, :]
        cp[:, OFF_W1T : OFF_W1T + 128] = w1t[0:128, :]
        cp[:, OFF_W1T + 128 : OFF_W1T + 256] = w1t[128:256, :]
        cp[:, OFF_W2T : OFF_W2T + 256] = w2t[0:128, :]
        cp[:, OFF_W2T + 256 : OFF_W2T + 512] = w2t[128:256, :]
        cp[:, OFF_B1] = np.concatenate([b1, b1])
        cp[:, OFF_B2] = b2[0:128]
        cp[:, OFF_B2 + 1] = b2[128:256]
        cp[:, OFF_EM : OFF_EM + NJ] = em[None, :]
        cp[:, OFF_BIASE : OFF_BIASE + NJ] = (BIG * (1.0 - em))[None, :]
        cp[:, OFF_BIASEN : OFF_BIASEN + NJ] = (-BIG * (1.0 - em))[None, :]
        cp[:, OFF_RECIPE] = 1.0 / em.sum()
        in_maps.append(dict(d=dT, cpack=cp))
    return in_maps


def _assemble(results):
    out = np.empty((4, 16, 256), dtype=np.float32)
    for k in range(8):
        b, ih = k // 2, k % 2
        out[b, ih * 8 : (ih + 1) * 8, :] = results[k]["out_t"].T
    return out


def run(trace=False, **inputs):
    from concourse.bass_utils import run_bass_kernel_spmd

    nc = _get_program()
    in_maps = _make_in_maps(**inputs)
    res = run_bass_kernel_spmd(
        nc, in_maps, core_ids=list(range(8)), trace=trace
    )
    return _assemble(res.results), res


def kernel(**inputs):
    out, _ = run(trace=False, **inputs)
    return out
